# revision 61
# baseline (speedup 1.0000x reference)
"""Fused causal multi-head attention on 8 Trainium2 NeuronCores.

Problem: x[4,2048,1024], W_qkv[3072,1024], W_out[1024,1024], NH=16 heads,
HD=64, causal softmax attention + output projection (fp32 reference).

Sharding: core c = 2*b + g handles batch b (of 4) and head-group g (of 2,
8 heads each).  Each core computes Q/K/V for its heads from x[b], runs
causal attention, and multiplies its half of the attention features into
W_out, producing a partial y[b] (full feature width, bf16).  The host
unshards by summing the two partial results per batch (standard
tensor-parallel output reduce) and concatenating over batches.

Kernel notes:
 - matmul operands are bf16 (full PE rate + fast weight load); every
   accumulation is fp32 in PSUM; softmax stats (exp input, sums,
   reciprocal) are fp32.
 - scores are computed transposed: S.T[k,q] = K_blk.T-matmul so the
   softmax denominator comes free via a ones-column appended to V and no
   PE transposes of the attention matrix are needed.
 - softmax skips max-subtraction (scores are ~N(0,1) by construction;
   exp stays well inside fp32 range).  Causal masking is multiplicative
   {0,1} applied after exp - identical result to the reference's
   additive -1e9 mask.  The last k-group of each q-chunk is >= half
   above the diagonal, so only its valid q-half is computed.
 - S.T matmuls come in same-shape pairs with one wide exp over a 2-bank
   PSUM super-tile (amortizes ACT overhead, avoids PE stationary-shape
   flips).
 - the PE is kept saturated through the attention phase by interleaving
   independent full-array work between attention groups via per-pair
   injection schedules: pair 0 absorbs the V tail + pair 1's projection
   chains, pair 1 absorbs pairs 2/3's, pair 2 absorbs the pairs-0+1
   output-projection chains, and pair 3 (heads 6+7 interleaved per
   q-chunk, per-chunk normalization) absorbs the remaining y chains as
   its own filler, each chunk's popping during the next chunk's groups.
   PE duty near 100% in every phase keeps the HAM clock gate at 2.4 GHz
   (a few us of PE idle drops the whole core to 1.2 GHz with long
   hysteresis).
 - output projection is PSUM-paired: pairs 0+1 accumulate in PSUM and
   land in an SBUF bf16 accumulator with one DVE copy; pairs 2+3
   accumulate in PSUM on top and finish with one DVE add + the output
   DMA.  HBM sees one [H,T] bf16 write instead of four fp32 partials
   and the y DVE traffic is half of the naive per-pair scheme.
 - inputs stream on all three DGE queues (weights on SP, x chunks on
   Activation + GpSimd) so the first projection chains start ~10us in.
 - normalization: fp32 reciprocal_approx_fast of the staged sums rows
   (DVE), partition-broadcast on GpSimd, multiply on DVE.  A
   pair-ending head's normalize is deferred past the next head's first
   q-chunk: its DVE work overlaps mask-free groups instead of stalling
   the new pair's first mask-multiplies.
"""

import sys

sys.path.insert(0, "/opt/trn_rl_repo")

import numpy as np

B, T, H = 4, 2048, 1024
NH, HD = 16, 64
NCORES = 8
NHL = NH // 2          # local heads per core = 8
CW = NHL * HD          # local attention feature width = 512
TCH = 512              # t-chunk (qkv, q-chunks, y)
NT = T // TCH          # 4
KB = 128               # k block rows
NKB = T // KB          # 16
VSEG = HD + 1          # V columns + ones column = 65


def _imports():
    global bass, bacc, mybir, tile, F32, BF16, ExitStack
    import concourse.bass as bass
    import concourse.bacc as bacc
    import concourse.mybir as mybir
    from concourse import tile
    from contextlib import ExitStack
    F32 = mybir.dt.float32
    BF16 = mybir.dt.bfloat16


def build_nc():
    """Build + compile the single-core SPMD Bass program."""
    _imports()
    nc = bacc.Bacc("TRN2", target_bir_lowering=False, debug=False,
                   num_devices=NCORES)

    xT = nc.dram_tensor("xT", [H, T], BF16, kind="ExternalInput").ap()
    wqkT = nc.dram_tensor("wqkT", [H, 2 * CW], BF16, kind="ExternalInput").ap()
    wvT = nc.dram_tensor("wvT", [H, CW], BF16, kind="ExternalInput").ap()
    woT = nc.dram_tensor("woT", [CW, H], BF16, kind="ExternalInput").ap()
    masks = nc.dram_tensor("masks", [128, 3 * TCH], BF16,
                           kind="ExternalInput").ap()
    yP = nc.dram_tensor("yP", [H, T], BF16, kind="ExternalOutput").ap()

    HC = H // 128  # 8 contraction chunks over the model dim

    with tile.TileContext(nc) as tc, ExitStack() as ctx, \
            nc.allow_low_precision(reason="bf16 matmul operands, fp32 accum"):
        mm = nc.tensor.matmul
        const = ctx.enter_context(tc.tile_pool(name="const", bufs=1))
        wpool = ctx.enter_context(tc.tile_pool(name="wpool", bufs=8))
        wop = ctx.enter_context(tc.tile_pool(name="wop", bufs=4))
        qa = ctx.enter_context(tc.tile_pool(name="qa", bufs=5))
        ktp = ctx.enter_context(tc.tile_pool(name="ktp", bufs=4))
        vp = ctx.enter_context(tc.tile_pool(name="vp", bufs=1))
        xp = ctx.enter_context(tc.tile_pool(name="xp", bufs=8))
        pts = ctx.enter_context(tc.tile_pool(name="pts", bufs=4))
        accp = ctx.enter_context(tc.tile_pool(name="accp", bufs=8))
        sm = ctx.enter_context(tc.tile_pool(name="sm", bufs=2))
        psum = ctx.enter_context(tc.tile_pool(name="psum", bufs=1, space="PSUM"))

        # ---- weights + activations.  Per-DMA queue-issue cost is
        # ~0.6-0.8us, so the first wave (wv + x chunk 0, consumed
        # hc-ascending by the first V chain) is interleaved across all
        # three DGE queues to minimize the latest-request time; wqk
        # follows on SP, later x chunks on Activation/GpSimd, and wo +
        # masks (not needed until attention) load last.
        wv = [wpool.tile([128, CW], BF16, tag="wv", name=f"wv{hc}")
              for hc in range(HC)]
        xt = [xp.tile([128, T], BF16, tag="xp", name=f"xt{hc}")
              for hc in range(HC)]

        def _wv_dma(eng, hc):
            eng.dma_start(wv[hc][:], wvT[hc * 128:(hc + 1) * 128, :])

        def _xt_dma(eng, hc, tci):
            ts_ = slice(tci * TCH, (tci + 1) * TCH)
            eng.dma_start(xt[hc][:, ts_], xT[hc * 128:(hc + 1) * 128, ts_])

        # first wave, roughly arrival-ordered for hc-ascending consumption
        for eng, op, hc in [
                (nc.sync, 'wv', 0), (nc.scalar, 'xt', 0), (nc.gpsimd, 'xt', 4),
                (nc.sync, 'wv', 1), (nc.scalar, 'xt', 1), (nc.gpsimd, 'xt', 5),
                (nc.sync, 'wv', 2), (nc.scalar, 'xt', 2), (nc.gpsimd, 'wv', 5),
                (nc.sync, 'xt', 6), (nc.scalar, 'xt', 3), (nc.gpsimd, 'wv', 6),
                (nc.sync, 'xt', 7), (nc.scalar, 'wv', 3), (nc.gpsimd, 'wv', 7),
                (nc.scalar, 'wv', 4)]:
            if op == 'wv':
                _wv_dma(eng, hc)
            else:
                _xt_dma(eng, hc, 0)
        wqk = []
        for hc in range(HC):
            w = wpool.tile([128, 2 * CW], BF16, tag="w", name=f"wqk{hc}")
            nc.sync.dma_start(w[:], wqkT[hc * 128:(hc + 1) * 128, :])
            wqk.append(w)
        for tci in range(1, NT):
            eng = nc.scalar if tci == 1 else nc.gpsimd
            for hc in range(HC):
                _xt_dma(eng, hc, tci)
        wo = []
        for cc in range(4):
            w = wop.tile([128, H], BF16, tag="wo", name=f"wo{cc}")
            nc.gpsimd.dma_start(w[:], woT[cc * 128:(cc + 1) * 128, :])
            wo.append(w)
        mask_t = []
        m0 = const.tile([128, 896], BF16, tag="mask0", name="mask0")
        nc.sync.dma_start(m0[:], masks[:, 0:896])
        mask_t.append(m0)
        m1 = const.tile([128, TCH], BF16, tag="mask1", name="mask1")
        nc.sync.dma_start(m1[:], masks[:, 896:896 + TCH])
        mask_t.append(m1)

        QT = [qa.tile([128, T], BF16, tag="qa", name=f"QT{i}") for i in range(4)]
        KT = [ktp.tile([128, T], BF16, tag="kt", name=f"KT{i}") for i in range(4)]
        # V, bf16, [t-block, head-major 65-wide segments (64 dims + ones col)]
        V = vp.tile([128, NKB * NHL * VSEG], BF16, name="Vsb")
        Vr = V[:].rearrange("p (tb h s) -> p tb h s", h=NHL, s=VSEG)
        # all ones columns in one strided memset
        nc.vector.memset(Vr[:, :, :, HD:VSEG], 1.0)
        # y accumulator, bf16 [f-block, T]
        acc = [accp.tile([128, T], BF16, tag="acc", name=f"acc{f}")
               for f in range(8)]

        # ---- chain emitters ----
        def qk_chain(r, tci):
            # QK projection chain for row-block r (pair r%4; q if r<4 else k)
            def emit():
                ts_ = slice(tci * TCH, (tci + 1) * TCH)
                ps = psum.tile([128, TCH], F32, tag="ps_qk", bufs=2,
                               name=f"psqk{r}_{tci}")
                for hc in range(HC):
                    mm(ps[:], wqk[hc][:, r * 128:(r + 1) * 128],
                       xt[hc][:, ts_], start=(hc == 0), stop=(hc == HC - 1))
                dst = QT[r] if r < 4 else KT[r - 4]
                nc.vector.tensor_copy(dst[:, ts_], ps[:])
            return emit

        def v_chain(tb):
            # V projection for t-block tb -> V sbuf (ones cols pre-set)
            def emit():
                tci, tbl = tb // 4, tb % 4
                pv = psum.tile([128, CW], F32, tag="ps_qk", bufs=2,
                               name=f"psv{tb}")
                for hc in range(HC):
                    mm(pv[:], xt[hc][:, tci * TCH + tbl * 128:
                                     tci * TCH + (tbl + 1) * 128],
                       wv[hc][:], start=(hc == 0), stop=(hc == HC - 1))
                src = pv[:].rearrange("p (h d) -> p h d", d=HD)
                nc.vector.tensor_copy(Vr[:, tb, :, 0:HD], src)
            return emit

        attnT = []

        def y01_chain(f, tci):
            # pairs 0+1 output-projection partial: both accumulate in PSUM
            # (one DVE copy instead of a copy + an add)
            def emit():
                ts_ = slice(tci * TCH, (tci + 1) * TCH)
                py = psum.tile([128, TCH], F32, tag="ps_qk", bufs=2,
                               name=f"psy01_{f}_{tci}")
                mm(py[:], wo[0][:, f * 128:(f + 1) * 128],
                   attnT[0][:, ts_], start=True, stop=False)
                mm(py[:], wo[1][:, f * 128:(f + 1) * 128],
                   attnT[1][:, ts_], start=False, stop=True)
                nc.vector.tensor_copy(acc[f][:, ts_], py[:])
            return emit

        def y012_chain(f, tci):
            # pairs 0-2 partials for the last t-chunk: all three accumulate
            # in PSUM (every dependency is a full pair old), one DVE copy
            def emit():
                ts_ = slice(tci * TCH, (tci + 1) * TCH)
                py = psum.tile([128, TCH], F32, tag="ps_qk", bufs=2,
                               name=f"psy012_{f}_{tci}")
                for cc in range(3):
                    mm(py[:], wo[cc][:, f * 128:(f + 1) * 128],
                       attnT[cc][:, ts_], start=(cc == 0), stop=(cc == 2))
                nc.vector.tensor_copy(acc[f][:, ts_], py[:])
            return emit

        def y3_chain(f, tci):
            # last pair's partial alone: one matmul in the tail, then the
            # final add + output DMA
            def emit():
                ts_ = slice(tci * TCH, (tci + 1) * TCH)
                py = psum.tile([128, TCH], F32, tag="ps_qk", bufs=2,
                               name=f"psy3_{f}_{tci}")
                mm(py[:], wo[3][:, f * 128:(f + 1) * 128],
                   attnT[3][:, ts_], start=True, stop=True)
                nc.vector.tensor_add(acc[f][:, ts_], acc[f][:, ts_], py[:])
                nc.sync.dma_start(yP[f * 128:(f + 1) * 128, ts_],
                                  acc[f][:, ts_])
            return emit

        def y23_chain(f, tci):
            # pairs 2+3 partial on top: PSUM-accumulated, one DVE add, then
            # the final output DMA for this (f, t-chunk)
            def emit():
                ts_ = slice(tci * TCH, (tci + 1) * TCH)
                py = psum.tile([128, TCH], F32, tag="ps_qk", bufs=2,
                               name=f"psy23_{f}_{tci}")
                mm(py[:], wo[2][:, f * 128:(f + 1) * 128],
                   attnT[2][:, ts_], start=True, stop=False)
                mm(py[:], wo[3][:, f * 128:(f + 1) * 128],
                   attnT[3][:, ts_], start=False, stop=True)
                nc.vector.tensor_add(acc[f][:, ts_], acc[f][:, ts_], py[:])
                nc.sync.dma_start(yP[f * 128:(f + 1) * 128, ts_],
                                  acc[f][:, ts_])
            return emit

        def attn_group(h, qci, g, ob):
            # one attention k-group: paired S.T matmuls, wide exp, mask,
            # PV accumulation
            p, off = h // 2, 64 * (h % 2)
            nkb = 4 * (qci + 1)
            ngrp = nkb // 2
            kb0, kb1 = 2 * g, 2 * g + 1
            dg = g - (ngrp - 2)
            # last group of each q-chunk is >= half above the causal
            # diagonal: compute only its valid q half [256:512)
            w_ = TCH if dg < 1 else TCH // 2
            q0 = 0 if dg < 1 else TCH // 2
            w1 = TCH - 128 if dg == 0 else w_
            q1 = 128 if dg == 0 else q0
            qsl = slice(qci * TCH + q0, (qci + 1) * TCH)
            qsl1 = slice(qci * TCH + q1, (qci + 1) * TCH)
            sb = psum.tile([128, w_ + w1], F32, tag="ps_s", bufs=2,
                           name=f"sb{h}_{qci}_{g}")
            mm(sb[:, 0:w_], KT[p][off:off + 64, kb0 * KB:(kb0 + 1) * KB],
               QT[p][off:off + 64, qsl], start=True, stop=True)
            mm(sb[:, w_:w_ + w1],
               KT[p][off:off + 64, kb1 * KB:(kb1 + 1) * KB],
               QT[p][off:off + 64, qsl1], start=True, stop=True)
            pt = pts.tile([128, w_ + w1], BF16, tag="pts",
                          name=f"pt{h}_{qci}_{g}")
            nc.scalar.activation(pt[:], sb[:],
                                 mybir.ActivationFunctionType.Exp)
            if dg >= 0:
                nc.vector.tensor_mul(pt[:], pt[:], mask_t[dg][:])
            mm(ob[0:VSEG, q0:TCH], Vr[:, kb0, h, :], pt[:, 0:w_],
               start=(kb0 == 0), stop=False)
            mm(ob[0:VSEG, q1:TCH], Vr[:, kb1, h, :], pt[:, w_:w_ + w1],
               start=False, stop=(kb1 == nkb - 1))

        # ======= up-front projections (DMA-arrival order); only what
        # head 0's first two q-chunks need - the rest pops during pair 0
        # at full clock instead of crawling through the half-clock,
        # DMA-bound ramp =======
        for tb in range(4):
            v_chain(tb)()
        qk_chain(0, 0)()
        qk_chain(4, 0)()
        for tb in range(4, 8):
            v_chain(tb)()
        qk_chain(0, 1)()
        qk_chain(4, 1)()

        # per-pair injection schedules: spread independent PE work evenly so
        # every pair keeps the duty monitor at full clock.  Pair 0's list is
        # ordered so its own later q-chunk dependencies pop in time.
        fills = [
            [qk_chain(0, 2), qk_chain(4, 2), v_chain(8), v_chain(9),
             qk_chain(0, 3), qk_chain(4, 3), v_chain(10), v_chain(11)]
            + [v_chain(tb) for tb in range(12, 16)]
            + [qk_chain(r, tci) for tci in range(NT) for r in (1, 5)],
            [qk_chain(r, tci) for tci in range(NT) for r in (2, 6)]
            + [qk_chain(r, tci) for tci in range(2) for r in (3, 7)],
            [qk_chain(r, tci) for tci in range(2, NT) for r in (3, 7)]
            + [y01_chain(f, tci) for tci in range(2) for f in range(8)],
        ]
        # pair 3's own early fill: the last two t-chunks' y01 chains (its
        # first chunks are thin - 4/8 attention groups - and need the work)
        p3_fill = [y01_chain(f, tci) for tci in (2, 3) for f in range(8)]

        def batched_norm(h, at, stage):
            # one batched approx reciprocal for the 4 staged sums rows, then
            # broadcast (GpSimd) + in-place normalize (DVE)
            off = 64 * (h % 2)
            def emit():
                nc.vector.reciprocal_approx_fast(stage[:], stage[:])
                for qq in range(NT):
                    rc0 = sm.tile([1, TCH], F32, tag="rc0",
                                  name=f"rc0_{h}_{qq}")
                    nc.sync.dma_start(rc0[:], stage[32 * qq:32 * qq + 1, :])
                    bcs = sm.tile([128, TCH], F32, tag="bcs",
                                  name=f"bcs{h}_{qq}")
                    nc.gpsimd.partition_broadcast(bcs[:], rc0[:], channels=128)
                    nc.vector.tensor_mul(
                        at[off:off + 64, qq * TCH:(qq + 1) * TCH],
                        at[off:off + 64, qq * TCH:(qq + 1) * TCH],
                        bcs[off:off + 64, :])
            return emit

        # ============ pairs 0-2: head-sequential attention ============
        # A pair-ending head's normalize is only read by the NEXT pair's y
        # chains, so it is deferred past the next head's first q-chunk: its
        # DVE work then overlaps mask-free attention groups instead of
        # stalling the new pair's first mask-multiplies.
        deferred_norm = None
        for h in range(6):
            p, off = h // 2, 64 * (h % 2)
            fill = fills[p]
            if h % 2 == 0:
                a = qa.tile([128, T], BF16, tag="qa", name=f"attnT{p}")
                attnT.append(a)
            at = attnT[p]
            # sums staging: one row per q-chunk at 32-partition offsets
            stage = sm.tile([128, TCH], F32, tag="stg", name=f"stg{h}")
            nc.any.memset(stage[:], 1.0)
            for qci in range(NT):
                qs = slice(qci * TCH, (qci + 1) * TCH)
                ob = psum.tile([128, TCH], F32, tag="ps_ob", bufs=2,
                               name=f"ob{h}_{qci}")
                for g in range(2 * (qci + 1)):
                    attn_group(h, qci, g, ob)
                    if fill:
                        fill.pop(0)()
                # evict unnormalized rows + stage the sums row; normalization
                # is batched at the head boundary (one reciprocal for 4
                # q-chunks)
                nc.vector.tensor_copy(at[off:off + 64, qs], ob[0:64, :])
                nc.vector.tensor_copy(stage[32 * qci:32 * qci + 1, :],
                                      ob[64:65, :])
                if qci == 0 and deferred_norm is not None:
                    deferred_norm()
                    deferred_norm = None
            if h % 2 == 0:
                batched_norm(h, at, stage)()
            else:
                deferred_norm = batched_norm(h, at, stage)

        # ====== pair 3: heads 6+7 interleaved per q-chunk with per-chunk
        # normalize; each chunk's y chains pop during the NEXT chunk's
        # groups (their normalize dependency is then a full chunk old, so
        # they never stall the PE) ======
        a = qa.tile([128, T], BF16, tag="qa", name="attnT3")
        attnT.append(a)
        at = attnT[3]
        for t_ in fills[2]:   # leftovers (pair 2 has 40 slots for 36)
            t_()
        inline = list(p3_fill)
        for qci in range(NT):
            qs = slice(qci * TCH, (qci + 1) * TCH)
            for h in (6, 7):
                off = 64 * (h % 2)
                ob = psum.tile([128, TCH], F32, tag="ps_ob", bufs=2,
                               name=f"ob{h}_{qci}")
                for g in range(2 * (qci + 1)):
                    attn_group(h, qci, g, ob)
                    if inline:
                        inline.pop(0)()
                nc.vector.tensor_copy(at[off:off + 64, qs], ob[0:64, :])
                rc = sm.tile([1, TCH], F32, tag="rc0", name=f"rc{h}_{qci}")
                nc.vector.tensor_copy(rc[:], ob[64:65, :])
                nc.vector.reciprocal_approx_fast(rc[:], rc[:])
                bcs = sm.tile([128, TCH], F32, tag="bcs", name=f"bcs{h}_{qci}")
                nc.gpsimd.partition_broadcast(bcs[:], rc[:], channels=128)
                nc.vector.tensor_mul(at[off:off + 64, qs],
                                     at[off:off + 64, qs],
                                     bcs[off:off + 64, :])
                if qci == 0 and h == 6 and deferred_norm is not None:
                    deferred_norm()
                    deferred_norm = None
            inline.extend(y23_chain(f, qci) for f in range(8))

        # tail: drain the last chunk's y chains
        for t_ in inline:
            t_()

    nc.compile()
    return nc


def make_in_maps(x, W_qkv, W_out):
    """Host-side shard prep: per-core input dict (bf16 operands)."""
    import ml_dtypes
    bf16 = ml_dtypes.bfloat16
    x = np.asarray(x, np.float32)
    W_qkv = np.asarray(W_qkv, np.float32)
    W_out = np.asarray(W_out, np.float32)
    Wq, Wk, Wv = W_qkv[0:H], W_qkv[H:2 * H], W_qkv[2 * H:3 * H]
    scale = np.float32(1.0 / np.sqrt(HD))
    kk, qq = np.meshgrid(np.arange(128), np.arange(TCH), indexing="ij")
    pat = [(qq >= j * 128 + kk).astype(np.float32) for j in range(4)]
    masks = np.concatenate(
        [pat[0], pat[1][:, 128:], pat[2][:, TCH // 2:],
         pat[3][:, TCH // 2:], np.zeros((128, 128), np.float32)],
        axis=1).astype(bf16)
    in_maps = []
    for c in range(NCORES):
        b, g = c // 2, c % 2
        rows = slice(g * CW, (g + 1) * CW)
        in_maps.append({
            "xT": np.ascontiguousarray(x[b].T).astype(bf16),
            "wqkT": np.ascontiguousarray(
                np.concatenate([Wq[rows] * scale, Wk[rows]], axis=0).T
            ).astype(bf16),
            "wvT": np.ascontiguousarray(Wv[rows].T).astype(bf16),
            "woT": np.ascontiguousarray(W_out[:, rows].T).astype(bf16),
            "masks": masks,
        })
    return in_maps


def gather_output(results):
    """results: per-core dicts with 'yP' [H, T] bf16 partials -> [B,T,H]."""
    out = np.empty((B, T, H), np.float32)
    for b in range(B):
        acc = results[2 * b]["yP"].astype(np.float32)
        acc += results[2 * b + 1]["yP"].astype(np.float32)
        out[b] = acc.T
    return out


_CACHE = {}


def kernel(x, W_qkv, W_out):
    from concourse.bass_utils import run_bass_kernel_spmd
    if "nc" not in _CACHE:
        _CACHE["nc"] = build_nc()
    nc = _CACHE["nc"]
    in_maps = make_in_maps(x, W_qkv, W_out)
    res = run_bass_kernel_spmd(nc, in_maps, list(range(NCORES)))
    return gather_output(res.results)
